# revision 22
# baseline (speedup 1.0000x reference)
"""GATv2 (3-layer, heads=1) fully on Trainium2, 8 NeuronCores, ONE launch.

Sharding: destination-node ranges (6272 nodes/core) -> segment softmax and
scatter-add are core-local. Per layer the small xl table ([N,64]) is
AllGathered (bf16) and each core gathers xl[src] for its edges from
feature-transposed SBUF tables via the native Pool indirect_copy
instruction (8 per-source-core tables, half-split to satisfy the ~4096
index-offset limit). xr[dst] expansion and the segment num/den reductions
are one-hot matmuls on the PE. Host does only index prep + final pooling
combine.
"""
import sys
import numpy as np
import ml_dtypes

sys.path.insert(0, "/opt/trn_rl_repo")

import concourse.bass as bass
import concourse.mybir as mybir
from concourse.tile import TileContext
from concourse.masks import make_identity
import concourse.tile_sem_assignment as _tsa
# this walrus build allows very few semaphore waits per instruction;
# use few DMA sem lanes and legalize the rest onto NOP chains below.
_tsa.NUM_SWDGE_GLOBAL_SEMS = 2
_tsa.NUM_HWDGE_SEMS = 2
from concourse.bass_utils import run_bass_kernel_spmd

F32 = mybir.dt.float32
BF16 = mybir.dt.bfloat16
I16 = mybir.dt.int16
I8 = mybir.dt.int8
U16 = mybir.dt.uint16
AF = mybir.ActivationFunctionType
OP = mybir.AluOpType

NC_ = 8
N = 50000
DIN = 128
HID = 64
NG = 256
NEG = 0.2
RANGE = 6272              # dst nodes per core
NPAD = RANGE * NC_        # 50176
NB = RANGE // 128         # 49 blocks per core
HALF = RANGE // 2         # 3136 table columns (half-split)
NBIN = NC_ + 1            # 8 source-core bins + 1 self-loop bin
DINP = [DIN, HID, HID]


def _legalize_waits(nc, keep=1, nop_cap=1):
    """Move excess semaphore waits onto chained same-engine NOPs."""
    cnt = [0]

    def mknop(engine, waits):
        cnt[0] += 1
        n = mybir.InstNoOp(name=f"lgl-{cnt[0]}", ins=[], outs=[])
        n.engine = engine
        n.sync_info = mybir.SyncInfo(on_wait=list(waits), on_update=[])
        try:
            nc.register_instruction(n)
        except Exception:
            pass
        return n

    for bbname, bassbb in nc.bb_map.items():
        bb = bassbb.bb
        insts = bb.instructions
        out = []
        for inst in insts:
            si = inst.sync_info
            waits = list(si.on_wait) if si is not None else []
            if len(waits) > keep:
                excess, kept = waits[:-keep], waits[-keep:]
                for i in range(0, len(excess), nop_cap):
                    out.append(mknop(inst.engine, excess[i:i + nop_cap]))
                inst.sync_info = mybir.SyncInfo(on_wait=kept,
                                                on_update=list(si.on_update))
            out.append(inst)
        if len(out) != len(insts):
            bb.instructions = out


_CACHE = {}


def _build_program(meta):
    """meta = (tuple NCH[49], tuple caps[49*8]) — static across cores."""
    import os
    _skip_gather = os.environ.get("GAT_SKIP_GATHER") == "1"
    _skip_edge = os.environ.get("GAT_SKIP_EDGE") == "1"
    _nlayers = int(os.environ.get("GAT_LAYERS", "3"))
    key = ("prog", meta, _skip_gather, _skip_edge, _nlayers)
    if key in _CACHE:
        return _CACHE[key]
    NCH = list(meta[0])
    caps = np.asarray(meta[1], np.int64).reshape(NB, NBIN)
    NVb = [n * 128 for n in NCH]
    CHTOT = sum(NCH)
    TOTCOL = sum(NVb) // 16
    NCHMAX = max(NCH)
    # slot/idx col offsets
    soff = np.concatenate([[0], np.cumsum(NVb)])       # slot offset per block
    cb = np.zeros((NB, NBIN), np.int64)                # idx col base per call
    acc = 0
    for b in range(NB):
        for k in range(NBIN):
            cb[b, k] = acc
            acc += caps[b, k] // 16
    assert acc == TOTCOL

    nc = bass.Bass(num_devices=NC_)
    xT = nc.declare_dram_parameter("xT", [DIN, RANGE], BF16, isOutput=False)
    idxp = nc.declare_dram_parameter("idxp", [16, TOTCOL], U16, isOutput=False)
    dstlp = nc.declare_dram_parameter("dstlp", [128, CHTOT], I16, isOutput=False)
    mp = nc.declare_dram_parameter("mp", [128, CHTOT], I8, isOutput=False)
    blp = nc.declare_dram_parameter("blp", [128, NB], F32, isOutput=False)
    wps, atps, bps = [], [], []
    for li in range(3):
        wps.append(nc.declare_dram_parameter(f"w{li}", [DINP[li], 192], BF16, isOutput=False))
        atps.append(nc.declare_dram_parameter(f"at{li}", [1, 64], F32, isOutput=False))
        bps.append(nc.declare_dram_parameter(f"bs{li}", [128, 64], F32, isOutput=False))
    outp = nc.declare_dram_parameter("outp", [128, 128], F32, isOutput=True)

    with TileContext(nc) as tc:
        with tc.tile_pool(name="wp", bufs=1) as wp, \
             tc.tile_pool(name="wl", bufs=1) as wl, \
             tc.tile_pool(name="gp", bufs=2) as gp, \
             tc.tile_pool(name="bk", bufs=1) as bk, \
             tc.tile_pool(name="sc", bufs=2) as sc, \
             tc.tile_pool(name="pA", bufs=2, space="PSUM") as pA, \
             tc.tile_pool(name="pB", bufs=2, space="PSUM") as pB, \
             tc.tile_pool(name="pC", bufs=2, space="PSUM") as pC, \
             tc.tile_pool(name="pD", bufs=2, space="PSUM") as pD, \
             tc.tile_pool(name="dr", bufs=2, space="DRAM") as dr:

            # ---- persistent tiles ----
            iota_i = wp.tile([128, 128], mybir.dt.int32, tag="ioi")
            nc.gpsimd.iota(iota_i[:], pattern=[[1, 128]], base=0, channel_multiplier=0)
            iota_f = wp.tile([128, 128], F32, tag="iof")
            nc.vector.tensor_copy(out=iota_f[:], in_=iota_i[:])
            iota2_i = wp.tile([128, 256], mybir.dt.int32, tag="io2i")
            nc.gpsimd.iota(iota2_i[:], pattern=[[1, 256]], base=0, channel_multiplier=0)
            iota2_f = wp.tile([128, 256], F32, tag="io2f")
            nc.vector.tensor_copy(out=iota2_f[:], in_=iota2_i[:])
            id_f = wp.tile([128, 128], F32, tag="idf")
            make_identity(nc, id_f[:])
            ones1 = wp.tile([1, 128], F32, tag="ones1")
            nc.vector.memset(ones1[:], 1.0)
            id_b = wp.tile([128, 128], BF16, tag="idb")
            make_identity(nc, id_b[:])

            idx_sb = wp.tile([128, TOTCOL], U16, tag="idx")
            for g in range(8):
                nc.sync.dma_start(out=idx_sb[16 * g:16 * (g + 1), :], in_=idxp[:, :])
            dstl_sb = wp.tile([128, CHTOT], I16, tag="dstl")
            nc.sync.dma_start(out=dstl_sb[:], in_=dstlp[:, :])
            m_sb = wp.tile([128, CHTOT], I8, tag="m")
            nc.sync.dma_start(out=m_sb[:], in_=mp[:, :])
            bl_sb = wp.tile([128, NB], F32, tag="bl")
            nc.sync.dma_start(out=bl_sb[:], in_=blp[:, :])

            hT = wp.tile([128, RANGE], BF16, tag="hT")
            nc.sync.dma_start(out=hT[:], in_=xT[:, :])
            xr_own = wp.tile([128, NB * 64], F32, tag="xro")
            res_own = wp.tile([128, NB * 64], F32, tag="rso")
            xlT_own = wp.tile([64, RANGE], BF16, tag="xlt")
            tbls = [wp.tile([128, HALF], BF16, tag=f"tbl{k}", name=f"tbl{k}")
                    for k in range(NC_)]
            tbl_self = wp.tile([128, HALF], BF16, tag="tblS", name="tblS")
            tbls.append(tbl_self)
            pooled = [wp.tile([128, 64], F32, tag=f"pl{i}", name=f"pl{i}")
                      for i in range(2)]
            nc.vector.memset(pooled[0][:], 0.0)
            nc.vector.memset(pooled[1][:], 0.0)

            for li in range(_nlayers):
                dinp = DINP[li]
                last = li == _nlayers - 1
                wc = wl.tile([dinp, 192], BF16, tag="wc")
                nc.sync.dma_start(out=wc[:], in_=wps[li][:, :])
                at1 = wl.tile([1, 64], F32, tag="at1")
                nc.sync.dma_start(out=at1[:], in_=atps[li][:, :])
                pat = pC.tile([128, 512], F32, tag="x512")
                nc.tensor.matmul(out=pat[:, 0:64], lhsT=ones1[:], rhs=at1[:],
                                 start=True, stop=True)
                at = wl.tile([128, 64], F32, tag="at")
                nc.scalar.copy(out=at[:], in_=pat[:, 0:64])
                bs = wl.tile([128, 64], F32, tag="bs")
                nc.sync.dma_start(out=bs[:], in_=bps[li][:, :])

                # ---- node linear: [xl|xr|res] = h @ [Wl^T|Wr^T|Rw^T] ----
                for b in range(NB):
                    lhs = hT[0:dinp, 128 * b:128 * (b + 1)]
                    p1 = pA.tile([128, 512], F32, tag="t512")
                    nc.tensor.matmul(out=p1[:, 0:128], lhsT=lhs, rhs=wc[:, 0:128],
                                     start=True, stop=True)
                    p2 = pC.tile([128, 512], F32, tag="x512")
                    nc.tensor.matmul(out=p2[:, 0:64], lhsT=lhs, rhs=wc[:, 128:192],
                                     start=True, stop=True)
                    nc.scalar.copy(out=xr_own[:, 64 * b:64 * (b + 1)], in_=p1[:, 64:128])
                    nc.scalar.copy(out=res_own[:, 64 * b:64 * (b + 1)], in_=p2[:, 0:64])
                    xlb = sc.tile([128, 64], BF16, tag="xlb")
                    nc.scalar.copy(out=xlb[:], in_=p1[:, 0:64])
                    pxt = pB.tile([128, 1024], BF16, tag="b1024")
                    nc.tensor.transpose(out=pxt[0:64, 0:128], in_=xlb[:], identity=id_b[:])
                    nc.scalar.copy(out=xlT_own[:, 128 * b:128 * (b + 1)], in_=pxt[0:64, 0:128])

                # ---- AllGather xl (bf16) and fill gather tables ----
                cc_in = dr.tile([64, RANGE], BF16, tag="cci")
                nc.sync.dma_start(out=cc_in[:], in_=xlT_own[:])
                cc_out = dr.tile([64 * NC_, RANGE], BF16, tag="cco", addr_space="Shared")
                nc.gpsimd.collective_compute(
                    "AllGather", OP.bypass,
                    replica_groups=[list(range(NC_))],
                    ins=[cc_in[:].opt()], outs=[cc_out[:].opt()])
                for k in range(NC_):
                    nc.sync.dma_start(out=tbls[k][0:64, :],
                                      in_=cc_out[64 * k:64 * k + 64, 0:HALF])
                    nc.sync.dma_start(out=tbls[k][64:128, :],
                                      in_=cc_out[64 * k:64 * k + 64, HALF:RANGE])
                nc.sync.dma_start(out=tbl_self[0:64, :], in_=xlT_own[:, 0:HALF])
                nc.sync.dma_start(out=tbl_self[64:128, :], in_=xlT_own[:, HALF:RANGE])

                # ---- edge phase, per dst block ----
                for b in range(NB):
                    nch = NCH[b]
                    nv = NVb[b]
                    gat = gp.tile([128, nv], BF16, tag="g")
                    off = 0
                    for k in range(NBIN):
                        cap = int(caps[b, k])
                        if cap == 0 or _skip_gather:
                            continue
                        nc.gpsimd.indirect_copy(
                            out=gat[:, off:off + cap], data=tbls[k][:],
                            idxs=idx_sb[:, int(cb[b, k]):int(cb[b, k]) + cap // 16],
                            i_know_ap_gather_is_preferred=True)
                        off += cap
                    ch0 = int(soff[b]) // 128
                    dstl_f = bk.tile([128, NCHMAX], F32, tag="dstlf")
                    nc.vector.tensor_copy(out=dstl_f[:, 0:nch],
                                          in_=dstl_sb[:, ch0:ch0 + nch])

                    S_all = bk.tile([128, NCHMAX * 128], F32, tag="sall")
                    gT = bk.tile([128, NCHMAX * 128], F32, tag="gt")
                    xlg = bk.tile([128, NCHMAX * 64], F32, tag="xlg")
                    xrg = bk.tile([128, NCHMAX * 64], F32, tag="xrg")
                    exl = bk.tile([128, NCHMAX], F32, tag="exl")
                    if not _skip_edge:
                        # one-hot S_ep for all chunks in ONE op:
                        # S[p, c, j] = (dstl[p, c] == iota[j])
                        nc.vector.tensor_tensor(
                            out=S_all[:].rearrange("p (c j) -> p c j", c=NCHMAX)[:, 0:nch, :],
                            in0=dstl_f[:, 0:nch].rearrange("p (c one) -> p c one", one=1)
                                .to_broadcast([128, nch, 128])[:],
                            in1=iota_f[:].rearrange("p (one j) -> p one j", one=1)
                                .to_broadcast([128, nch, 128])[:],
                            op=OP.is_equal)
                    for G0 in range(0, nch if not _skip_edge else 0, 8):
                        GN = min(8, nch - G0)
                        pgt = pB.tile([128, 1024], BF16, tag="b1024")
                        for j in range(GN):
                            c = G0 + j
                            nc.tensor.transpose(out=pgt[:, 128 * j:128 * (j + 1)],
                                                in_=gat[:, 128 * c:128 * (c + 1)],
                                                identity=id_b[:])
                        nc.scalar.copy(out=gT[:, 128 * G0:128 * (G0 + GN)],
                                       in_=pgt[:, 0:128 * GN])
                    for g0 in range(0, nch if not _skip_edge else 0, 4):
                        gn = min(4, nch - g0)
                        ps = pA.tile([128, 512], F32, tag="t512")
                        for j in range(gn):
                            c = g0 + j
                            nc.tensor.transpose(out=ps[:, 128 * j:128 * (j + 1)],
                                                in_=S_all[:, 128 * c:128 * (c + 1)],
                                                identity=id_f[:])
                        spe = sc.tile([128, 512], F32, tag="spe")
                        nc.scalar.copy(out=spe[:, 0:128 * gn], in_=ps[:, 0:128 * gn])
                        pxr = pC.tile([128, 512], F32, tag="x512")
                        for j in range(gn):
                            nc.tensor.matmul(out=pxr[:, 64 * j:64 * (j + 1)],
                                             lhsT=spe[:, 128 * j:128 * (j + 1)],
                                             rhs=xr_own[:, 64 * b:64 * (b + 1)],
                                             start=True, stop=True)
                        nc.scalar.copy(out=xrg[:, 64 * g0:64 * (g0 + gn)],
                                       in_=pxr[:, 0:64 * gn])
                    if not _skip_edge:
                        gT3 = gT[:].rearrange("p (c f) -> p c f", c=NCHMAX)
                        xlg3 = xlg[:].rearrange("p (c f) -> p c f", c=NCHMAX)
                        xrg3 = xrg[:].rearrange("p (c f) -> p c f", c=NCHMAX)
                        m3 = m_sb[:, ch0:ch0 + nch].rearrange(
                            "p (c one) -> p c one", one=1).to_broadcast([128, nch, 64])
                        # half-select xlg from gT (even/odd 64-col halves by m)
                        nc.vector.tensor_copy(out=xlg3[:, 0:nch, :],
                                              in_=gT3[:, 0:nch, 0:64])
                        nc.vector.copy_predicated(out=xlg3[:, 0:nch, :], mask=m3[:],
                                                  data=gT3[:, 0:nch, 64:128])
                        # e = xlg + xrg -> gT even halves
                        nc.vector.tensor_tensor(out=gT3[:, 0:nch, 0:64],
                                                in0=xlg3[:, 0:nch, :],
                                                in1=xrg3[:, 0:nch, :], op=OP.add)
                        # leaky -> odd halves
                        nc.scalar.activation(out=gT3[:, 0:nch, 64:128],
                                             in_=gT3[:, 0:nch, 0:64],
                                             func=AF.Prelu, alpha=NEG)
                        # * att -> even halves
                        nc.vector.tensor_tensor(out=gT3[:, 0:nch, 0:64],
                                                in0=gT3[:, 0:nch, 64:128],
                                                in1=at[:].rearrange(
                                                    "p (one f) -> p one f", one=1)
                                                    .to_broadcast([128, nch, 64])[:],
                                                op=OP.mult)
                        # logits
                        nc.vector.tensor_reduce(out=exl[:, 0:nch],
                                                in_=gT3[:, 0:nch, 0:64],
                                                axis=mybir.AxisListType.X, op=OP.add)
                        nc.vector.tensor_scalar(out=exl[:, 0:nch], in0=exl[:, 0:nch],
                                                scalar1=60.0, scalar2=None, op0=OP.min)
                        nc.scalar.activation(out=exl[:, 0:nch], in_=exl[:, 0:nch],
                                             func=AF.Exp)
                        # vals (ex*xlg | ex) into gT cols [0:65*nch]
                        from concourse.bass_types import AP as _AP
                        g0 = gT[:]
                        vbody = _AP(g0.tensor, g0.offset,
                                    [list(g0.ap[0]), [65, nch], [1, 64]])
                        vex = _AP(g0.tensor, g0.offset + 64,
                                  [list(g0.ap[0]), [65, nch], [1, 1]])
                        nc.vector.tensor_tensor(
                            out=vbody,
                            in0=xlg3[:, 0:nch, :],
                            in1=exl[:, 0:nch].rearrange("p (c one) -> p c one", one=1)
                                .to_broadcast([128, nch, 64])[:],
                            op=OP.mult)
                        nc.vector.tensor_copy(out=vex,
                                              in_=exl[:, 0:nch].rearrange(
                                                  "p (c one) -> p c one", one=1)[:])
                    nd = pD.tile([128, 65], F32, tag="nd")
                    for c in range(nch if not _skip_edge else 1):
                        nc.tensor.matmul(out=nd[:], lhsT=S_all[:, 128 * c:128 * (c + 1)],
                                         rhs=gT[:, 65 * c:65 * (c + 1)],
                                         start=(c == 0), stop=(c == nch - 1))
                    nds = sc.tile([128, 65], F32, tag="nds")
                    nc.scalar.copy(out=nds[:], in_=nd[:])
                    den = sc.tile([128, 1], F32, tag="den")
                    nc.vector.tensor_scalar(out=den[:], in0=nds[:, 64:65], scalar1=1e-30,
                                            scalar2=None, op0=OP.add)
                    rden = sc.tile([128, 1], F32, tag="rden")
                    nc.vector.reciprocal(out=rden[:], in_=den[:])
                    hb = sc.tile([128, 64], F32, tag="hb")
                    nc.vector.tensor_scalar(out=hb[:], in0=nds[:, 0:64], scalar1=rden[:],
                                            scalar2=None, op0=OP.mult)
                    nc.vector.tensor_tensor(out=hb[:], in0=hb[:],
                                            in1=res_own[:, 64 * b:64 * (b + 1)], op=OP.add)
                    nc.vector.tensor_tensor(out=hb[:], in0=hb[:], in1=bs[:], op=OP.add)
                    hf = sc.tile([128, 64], F32, tag="hf")
                    nc.scalar.activation(out=hf[:], in_=hb[:],
                                         func=AF.Relu if not last else AF.Copy)
                    if not last:
                        ph = pA.tile([128, 512], F32, tag="t512")
                        nc.tensor.transpose(out=ph[0:64, 0:128], in_=hf[:], identity=id_f[:])
                        nc.scalar.copy(out=hT[0:64, 128 * b:128 * (b + 1)], in_=ph[0:64, 0:128])
                    else:
                        B = sc.tile([128, 256], F32, tag="B")
                        nc.vector.tensor_tensor(
                            out=B[:], in0=bl_sb[:, b:b + 1].to_broadcast([128, 256])[:],
                            in1=iota2_f[:], op=OP.is_equal)
                        for i in range(2):
                            pp = pC.tile([128, 512], F32, tag="x512")
                            nc.tensor.matmul(out=pp[:, 0:64], lhsT=B[:, 128 * i:128 * (i + 1)],
                                             rhs=hf[:], start=True, stop=True)
                            nc.vector.tensor_tensor(out=pooled[i][:], in0=pooled[i][:],
                                                    in1=pp[:, 0:64], op=OP.add)

            nc.sync.dma_start(out=outp[:, 0:64], in_=pooled[0][:])
            nc.sync.dma_start(out=outp[:, 64:128], in_=pooled[1][:])

    _legalize_waits(nc)
    _CACHE[key] = nc
    return nc


def _prep(ei):
    """Host index prep. Returns (meta, per-core input dict pieces)."""
    key = ("prep", ei.tobytes()[:4096], int(ei.sum()))
    if key in _CACHE:
        return _CACHE[key]
    E = ei.shape[1]
    src = np.concatenate([ei[0], np.arange(N, dtype=np.int64)])
    dst = np.concatenate([ei[1], np.arange(N, dtype=np.int64)])
    kk = src // RANGE                  # source core bin
    kk[E:] = NC_                       # self-loop bin (gathers own xl table)
    gb = dst >> 7                      # global 128-node block (0..391)
    order = np.lexsort((kk, gb))
    src, dst, gb, kk = src[order], dst[order], gb[order], kk[order]
    core = gb // NB
    b = gb % NB
    gid = (core * NB + b) * NBIN + kk  # per (core, block, bin) group
    cnts = np.bincount(gid, minlength=NC_ * NB * NBIN).reshape(NC_, NB, NBIN)
    caps = cnts.max(axis=0)            # [NB, 9]
    caps = ((caps + 31) // 32) * 32   # 32: keep idx slices 4B-aligned
    nvb = caps.sum(axis=1)
    nvb_pad = ((nvb + 127) // 128) * 128
    nvb_pad = np.maximum(nvb_pad, 128)
    caps[:, NBIN - 1] += nvb_pad - nvb  # absorb tail pad into last bin
    NCH = (nvb_pad // 128).astype(np.int64)
    meta = (tuple(int(x) for x in NCH), tuple(int(x) for x in caps.reshape(-1)))

    soff = np.concatenate([[0], np.cumsum(nvb_pad)])
    capoff = np.zeros((NB, NBIN), np.int64)
    acc = 0
    cbase = np.zeros((NB, NBIN), np.int64)
    for bb in range(NB):
        o = 0
        for k in range(NBIN):
            capoff[bb, k] = o
            cbase[bb, k] = acc
            o += caps[bb, k]
            acc += caps[bb, k] // 16
    TOTCOL = int(acc)
    CHTOT = int(soff[-1] // 128)

    # per-edge placement
    gstart = np.concatenate([[0], np.cumsum(cnts.reshape(-1))])
    rank = np.arange(len(src)) - np.repeat(gstart[:-1], cnts.reshape(-1))
    srcl = src % RANGE
    jv = (srcl % HALF).astype(np.uint16)
    hv = (srcl // HALF).astype(np.int8)
    dlv = (dst & 127).astype(np.int16)

    idxs, dstls, ms = [], [], []
    for c in range(NC_):
        sel = core == c
        r = rank[sel]
        bb = b[sel]
        k = kk[sel]
        idx_t = np.zeros((16, TOTCOL), np.uint16)
        idx_t[r & 15, cbase[bb, k] + (r >> 4)] = jv[sel]
        dstl_t = np.full((128, CHTOT), -1, np.int16)
        m_t = np.zeros((128, CHTOT), np.int8)
        slot = soff[bb] + capoff[bb, k] + r
        dstl_t[slot & 127, slot >> 7] = dlv[sel]
        m_t[slot & 127, slot >> 7] = hv[sel]
        idxs.append(idx_t)
        dstls.append(dstl_t)
        ms.append(m_t)
    out = (meta, idxs, dstls, ms)
    _CACHE[key] = out
    return out


def kernel(**inputs):
    inp = {k: np.asarray(v) for k, v in inputs.items()}
    x = inp["x"].astype(np.float32)
    ei = inp["edge_index"].astype(np.int64)
    batch = inp["batch"].astype(np.int64)

    meta, idxs, dstls, ms = _prep(ei)
    NCHMAX = max(meta[0])
    nc = _build_program(meta)

    xpad = np.zeros((NPAD, DIN), np.float32)
    xpad[:N] = x
    blpad = np.full(NPAD, -1, np.int64)
    blpad[:N] = batch
    in_maps = []
    for c in range(NC_):
        d = {
            "xT": np.ascontiguousarray(xpad[c * RANGE:(c + 1) * RANGE].T).astype(ml_dtypes.bfloat16),
            "idxp": idxs[c], "dstlp": dstls[c], "mp": ms[c],
            "blp": np.ascontiguousarray(
                blpad[c * RANGE:(c + 1) * RANGE].reshape(NB, 128).T.astype(np.float32)),
        }
        for li in range(3):
            dinp = DINP[li]
            wcat = np.zeros((dinp, 192), np.float32)
            wcat[:, 0:64] = inp[f"Wl{li}"].astype(np.float32).T
            wcat[:, 64:128] = inp[f"Wr{li}"].astype(np.float32).T
            wcat[:, 128:192] = inp[f"Rw{li}"].astype(np.float32).T
            d[f"w{li}"] = wcat.astype(ml_dtypes.bfloat16)
            d[f"at{li}"] = inp[f"att{li}"].astype(np.float32).reshape(1, 64)
            d[f"bs{li}"] = np.tile(
                (inp[f"b{li}"] + inp[f"Rb{li}"]).astype(np.float32), (128, 1))
        in_maps.append(d)

    res = run_bass_kernel_spmd(nc, in_maps, list(range(NC_)))
    pooled = np.zeros((NG, HID), np.float32)
    for c in range(NC_):
        o = res.results[c]["outp"]
        pooled[0:128] += o[:, 0:64]
        pooled[128:256] += o[:, 64:128]
    cnt = np.maximum(np.bincount(batch, minlength=NG), 1).astype(np.float32)
    pooled /= cnt[:, None]
    out = pooled @ inp["Wf"].astype(np.float32).T + inp["bf"].astype(np.float32)[None, :]
    kernel.last_hw_ns = 0
    return out.reshape(NG, 1).astype(np.float32)


kernel.last_hw_ns = 0


# revision 23
# speedup vs baseline: 1.2637x; 1.2637x over previous
"""GATv2 (3-layer, heads=1) fully on Trainium2, 8 NeuronCores, ONE launch.

Sharding: destination-node ranges (6272 nodes/core) -> segment softmax and
scatter-add are core-local. Per layer the small xl table ([N,64]) is
AllGathered (bf16) and each core gathers xl[src] for its edges from
feature-transposed SBUF tables via the native Pool indirect_copy
instruction (8 per-source-core tables, half-split to satisfy the ~4096
index-offset limit). xr[dst] expansion and the segment num/den reductions
are one-hot matmuls on the PE. Host does only index prep + final pooling
combine.
"""
import sys
import numpy as np
import ml_dtypes

sys.path.insert(0, "/opt/trn_rl_repo")

import concourse.bass as bass
import concourse.mybir as mybir
from concourse.tile import TileContext
from concourse.masks import make_identity
import concourse.tile_sem_assignment as _tsa
# this walrus build allows very few semaphore waits per instruction;
# use few DMA sem lanes and legalize the rest onto NOP chains below.
_tsa.NUM_SWDGE_GLOBAL_SEMS = 2
_tsa.NUM_HWDGE_SEMS = 2
from concourse.bass_utils import run_bass_kernel_spmd

F32 = mybir.dt.float32
BF16 = mybir.dt.bfloat16
I16 = mybir.dt.int16
I8 = mybir.dt.int8
U16 = mybir.dt.uint16
AF = mybir.ActivationFunctionType
OP = mybir.AluOpType

NC_ = 8
N = 50000
DIN = 128
HID = 64
NG = 256
NEG = 0.2
RANGE = 6272              # dst nodes per core
NPAD = RANGE * NC_        # 50176
NB = RANGE // 128         # 49 blocks per core
HALF = RANGE // 2         # 3136 table columns (half-split)
NBIN = NC_ + 1            # 8 source-core bins + 1 self-loop bin
DINP = [DIN, HID, HID]


def _legalize_waits(nc, keep=1, nop_cap=1):
    """Move excess semaphore waits onto chained same-engine NOPs."""
    cnt = [0]

    def mknop(engine, waits):
        cnt[0] += 1
        n = mybir.InstNoOp(name=f"lgl-{cnt[0]}", ins=[], outs=[])
        n.engine = engine
        n.sync_info = mybir.SyncInfo(on_wait=list(waits), on_update=[])
        try:
            nc.register_instruction(n)
        except Exception:
            pass
        return n

    for bbname, bassbb in nc.bb_map.items():
        bb = bassbb.bb
        insts = bb.instructions
        out = []
        for inst in insts:
            si = inst.sync_info
            waits = list(si.on_wait) if si is not None else []
            if len(waits) > keep:
                excess, kept = waits[:-keep], waits[-keep:]
                for i in range(0, len(excess), nop_cap):
                    out.append(mknop(inst.engine, excess[i:i + nop_cap]))
                inst.sync_info = mybir.SyncInfo(on_wait=kept,
                                                on_update=list(si.on_update))
            out.append(inst)
        if len(out) != len(insts):
            bb.instructions = out


_CACHE = {}


def _build_program(meta):
    """meta = (tuple NCH[49], tuple caps[49*8]) — static across cores."""
    import os
    _skip_gather = os.environ.get("GAT_SKIP_GATHER") == "1"
    _skip_edge = os.environ.get("GAT_SKIP_EDGE") == "1"
    _nlayers = int(os.environ.get("GAT_LAYERS", "3"))
    key = ("prog", meta, _skip_gather, _skip_edge, _nlayers)
    if key in _CACHE:
        return _CACHE[key]
    NCH = list(meta[0])
    caps = np.asarray(meta[1], np.int64).reshape(NB, NBIN)
    NVb = [n * 128 for n in NCH]
    CHTOT = sum(NCH)
    TOTCOL = sum(NVb) // 16
    NCHMAX = max(NCH)
    # slot/idx col offsets
    soff = np.concatenate([[0], np.cumsum(NVb)])       # slot offset per block
    cb = np.zeros((NB, NBIN), np.int64)                # idx col base per call
    acc = 0
    for b in range(NB):
        for k in range(NBIN):
            cb[b, k] = acc
            acc += caps[b, k] // 16
    assert acc == TOTCOL

    nc = bass.Bass(num_devices=NC_)
    xT = nc.declare_dram_parameter("xT", [DIN, RANGE], BF16, isOutput=False)
    idxp = nc.declare_dram_parameter("idxp", [16, TOTCOL], U16, isOutput=False)
    dstlp = nc.declare_dram_parameter("dstlp", [128, CHTOT], I16, isOutput=False)
    mp = nc.declare_dram_parameter("mp", [128, CHTOT], I8, isOutput=False)
    blp = nc.declare_dram_parameter("blp", [128, NB], F32, isOutput=False)
    wps, atps, bps = [], [], []
    for li in range(3):
        wps.append(nc.declare_dram_parameter(f"w{li}", [DINP[li], 192], BF16, isOutput=False))
        atps.append(nc.declare_dram_parameter(f"at{li}", [1, 64], F32, isOutput=False))
        bps.append(nc.declare_dram_parameter(f"bs{li}", [128, 64], F32, isOutput=False))
    outp = nc.declare_dram_parameter("outp", [128, 128], F32, isOutput=True)

    with TileContext(nc) as tc:
        with tc.tile_pool(name="wp", bufs=1) as wp, \
             tc.tile_pool(name="wl", bufs=1) as wl, \
             tc.tile_pool(name="gp", bufs=2) as gp, \
             tc.tile_pool(name="bk", bufs=1) as bk, \
             tc.tile_pool(name="sc", bufs=2) as sc, \
             tc.tile_pool(name="pA", bufs=2, space="PSUM") as pA, \
             tc.tile_pool(name="pB", bufs=2, space="PSUM") as pB, \
             tc.tile_pool(name="pC", bufs=2, space="PSUM") as pC, \
             tc.tile_pool(name="pD", bufs=2, space="PSUM") as pD, \
             tc.tile_pool(name="dr", bufs=2, space="DRAM") as dr:

            # ---- persistent tiles ----
            iota_i = wp.tile([128, 128], mybir.dt.int32, tag="ioi")
            nc.gpsimd.iota(iota_i[:], pattern=[[1, 128]], base=0, channel_multiplier=0)
            iota_f = wp.tile([128, 128], F32, tag="iof")
            nc.vector.tensor_copy(out=iota_f[:], in_=iota_i[:])
            iota2_i = wp.tile([128, 256], mybir.dt.int32, tag="io2i")
            nc.gpsimd.iota(iota2_i[:], pattern=[[1, 256]], base=0, channel_multiplier=0)
            iota2_f = wp.tile([128, 256], F32, tag="io2f")
            nc.vector.tensor_copy(out=iota2_f[:], in_=iota2_i[:])
            id_f = wp.tile([128, 128], F32, tag="idf")
            make_identity(nc, id_f[:])
            ones1 = wp.tile([1, 128], F32, tag="ones1")
            nc.vector.memset(ones1[:], 1.0)
            id_b = wp.tile([128, 128], BF16, tag="idb")
            make_identity(nc, id_b[:])

            idx_sb = wp.tile([128, TOTCOL], U16, tag="idx")
            for g in range(8):
                nc.sync.dma_start(out=idx_sb[16 * g:16 * (g + 1), :], in_=idxp[:, :])
            dstl_sb = wp.tile([128, CHTOT], I16, tag="dstl")
            nc.sync.dma_start(out=dstl_sb[:], in_=dstlp[:, :])
            m_sb = wp.tile([128, CHTOT], I8, tag="m")
            nc.sync.dma_start(out=m_sb[:], in_=mp[:, :])
            bl_sb = wp.tile([128, NB], F32, tag="bl")
            nc.sync.dma_start(out=bl_sb[:], in_=blp[:, :])

            hT = wp.tile([128, RANGE], BF16, tag="hT")
            nc.sync.dma_start(out=hT[:], in_=xT[:, :])
            xr_own = wp.tile([128, NB * 64], F32, tag="xro")
            res_own = wp.tile([128, NB * 64], F32, tag="rso")
            xlT_own = wp.tile([64, RANGE], BF16, tag="xlt")
            tbls = [wp.tile([128, HALF], BF16, tag=f"tbl{k}", name=f"tbl{k}")
                    for k in range(NC_)]
            tbl_self = wp.tile([128, HALF], BF16, tag="tblS", name="tblS")
            tbls.append(tbl_self)
            pooled = [wp.tile([128, 64], F32, tag=f"pl{i}", name=f"pl{i}")
                      for i in range(2)]
            nc.vector.memset(pooled[0][:], 0.0)
            nc.vector.memset(pooled[1][:], 0.0)

            for li in range(_nlayers):
                dinp = DINP[li]
                last = li == _nlayers - 1
                wc = wl.tile([dinp, 192], BF16, tag="wc")
                nc.sync.dma_start(out=wc[:], in_=wps[li][:, :])
                at1 = wl.tile([1, 64], F32, tag="at1")
                nc.sync.dma_start(out=at1[:], in_=atps[li][:, :])
                pat = pC.tile([128, 512], F32, tag="x512")
                nc.tensor.matmul(out=pat[:, 0:64], lhsT=ones1[:], rhs=at1[:],
                                 start=True, stop=True)
                at = wl.tile([128, 64], F32, tag="at")
                nc.scalar.copy(out=at[:], in_=pat[:, 0:64])
                bs = wl.tile([128, 64], F32, tag="bs")
                nc.sync.dma_start(out=bs[:], in_=bps[li][:, :])

                # ---- node linear: [xl|xr|res] = h @ [Wl^T|Wr^T|Rw^T] ----
                for b in range(NB):
                    lhs = hT[0:dinp, 128 * b:128 * (b + 1)]
                    p1 = pA.tile([128, 512], F32, tag="t512")
                    nc.tensor.matmul(out=p1[:, 0:128], lhsT=lhs, rhs=wc[:, 0:128],
                                     start=True, stop=True)
                    p2 = pC.tile([128, 512], F32, tag="x512")
                    nc.tensor.matmul(out=p2[:, 0:64], lhsT=lhs, rhs=wc[:, 128:192],
                                     start=True, stop=True)
                    nc.scalar.copy(out=xr_own[:, 64 * b:64 * (b + 1)], in_=p1[:, 64:128])
                    nc.scalar.copy(out=res_own[:, 64 * b:64 * (b + 1)], in_=p2[:, 0:64])
                    xlb = sc.tile([128, 64], BF16, tag="xlb")
                    nc.scalar.copy(out=xlb[:], in_=p1[:, 0:64])
                    pxt = pB.tile([128, 1024], BF16, tag="b1024")
                    nc.tensor.transpose(out=pxt[0:64, 0:128], in_=xlb[:], identity=id_b[:])
                    nc.scalar.copy(out=xlT_own[:, 128 * b:128 * (b + 1)], in_=pxt[0:64, 0:128])

                # ---- AllGather xl (bf16) and fill gather tables ----
                cc_in = dr.tile([64, RANGE], BF16, tag="cci")
                nc.sync.dma_start(out=cc_in[:], in_=xlT_own[:])
                cc_out = dr.tile([64 * NC_, RANGE], BF16, tag="cco", addr_space="Shared")
                nc.gpsimd.collective_compute(
                    "AllGather", OP.bypass,
                    replica_groups=[list(range(NC_))],
                    ins=[cc_in[:].opt()], outs=[cc_out[:].opt()])
                for k in range(NC_):
                    nc.sync.dma_start(out=tbls[k][0:64, :],
                                      in_=cc_out[64 * k:64 * k + 64, 0:HALF])
                    nc.sync.dma_start(out=tbls[k][64:128, :],
                                      in_=cc_out[64 * k:64 * k + 64, HALF:RANGE])
                nc.sync.dma_start(out=tbl_self[0:64, :], in_=xlT_own[:, 0:HALF])
                nc.sync.dma_start(out=tbl_self[64:128, :], in_=xlT_own[:, HALF:RANGE])

                # ---- edge phase, per dst block ----
                for b in range(NB):
                    nch = NCH[b]
                    nv = NVb[b]
                    gat = gp.tile([128, nv], BF16, tag="g")
                    off = 0
                    for k in range(NBIN):
                        cap = int(caps[b, k])
                        if cap == 0 or _skip_gather:
                            continue
                        nc.gpsimd.indirect_copy(
                            out=gat[:, off:off + cap], data=tbls[k][:],
                            idxs=idx_sb[:, int(cb[b, k]):int(cb[b, k]) + cap // 16],
                            i_know_ap_gather_is_preferred=True)
                        off += cap
                    ch0 = int(soff[b]) // 128
                    dstl_f = bk.tile([128, NCHMAX], F32, tag="dstlf")
                    nc.vector.tensor_copy(out=dstl_f[:, 0:nch],
                                          in_=dstl_sb[:, ch0:ch0 + nch])

                    S_all = bk.tile([128, NCHMAX * 128], F32, tag="sall")
                    gT = bk.tile([128, NCHMAX * 128], F32, tag="gt")
                    xlg = bk.tile([128, NCHMAX * 64], F32, tag="xlg")
                    xrg = bk.tile([128, NCHMAX * 64], F32, tag="xrg")
                    exl = bk.tile([128, NCHMAX], F32, tag="exl")
                    if not _skip_edge:
                        # one-hot S_ep for all chunks in ONE op:
                        # S[p, c, j] = (dstl[p, c] == iota[j])
                        nc.vector.tensor_tensor(
                            out=S_all[:].rearrange("p (c j) -> p c j", c=NCHMAX)[:, 0:nch, :],
                            in0=dstl_f[:, 0:nch].rearrange("p (c one) -> p c one", one=1)
                                .to_broadcast([128, nch, 128])[:],
                            in1=iota_f[:].rearrange("p (one j) -> p one j", one=1)
                                .to_broadcast([128, nch, 128])[:],
                            op=OP.is_equal)
                    for G0 in range(0, nch if not _skip_edge else 0, 8):
                        GN = min(8, nch - G0)
                        pgt = pB.tile([128, 1024], BF16, tag="b1024")
                        for j in range(GN):
                            c = G0 + j
                            nc.tensor.transpose(out=pgt[:, 128 * j:128 * (j + 1)],
                                                in_=gat[:, 128 * c:128 * (c + 1)],
                                                identity=id_b[:])
                        nc.scalar.copy(out=gT[:, 128 * G0:128 * (G0 + GN)],
                                       in_=pgt[:, 0:128 * GN])
                    for G0 in range(0, nch if not _skip_edge else 0, 8):
                        GN = min(8, nch - G0)
                        pxr = pC.tile([128, 512], F32, tag="x512")
                        for g0 in range(G0, G0 + GN, 4):
                            gn = min(4, G0 + GN - g0)
                            ps = pA.tile([128, 512], F32, tag="t512")
                            for j in range(gn):
                                c = g0 + j
                                nc.tensor.transpose(out=ps[:, 128 * j:128 * (j + 1)],
                                                    in_=S_all[:, 128 * c:128 * (c + 1)],
                                                    identity=id_f[:])
                            spe = sc.tile([128, 512], F32, tag="spe")
                            nc.scalar.copy(out=spe[:, 0:128 * gn], in_=ps[:, 0:128 * gn])
                            for j in range(gn):
                                jj = g0 - G0 + j
                                nc.tensor.matmul(out=pxr[:, 64 * jj:64 * (jj + 1)],
                                                 lhsT=spe[:, 128 * j:128 * (j + 1)],
                                                 rhs=xr_own[:, 64 * b:64 * (b + 1)],
                                                 start=True, stop=True)
                        nc.scalar.copy(out=xrg[:, 64 * G0:64 * (G0 + GN)],
                                       in_=pxr[:, 0:64 * GN])
                    if not _skip_edge:
                        gT3 = gT[:].rearrange("p (c f) -> p c f", c=NCHMAX)
                        xlg3 = xlg[:].rearrange("p (c f) -> p c f", c=NCHMAX)
                        xrg3 = xrg[:].rearrange("p (c f) -> p c f", c=NCHMAX)
                        m3 = m_sb[:, ch0:ch0 + nch].rearrange(
                            "p (c one) -> p c one", one=1).to_broadcast([128, nch, 64])
                        # half-select xlg from gT (even/odd 64-col halves by m)
                        nc.vector.tensor_copy(out=xlg3[:, 0:nch, :],
                                              in_=gT3[:, 0:nch, 0:64])
                        nc.vector.copy_predicated(out=xlg3[:, 0:nch, :], mask=m3[:],
                                                  data=gT3[:, 0:nch, 64:128])
                        # e = xlg + xrg -> gT even halves
                        nc.vector.tensor_tensor(out=gT3[:, 0:nch, 0:64],
                                                in0=xlg3[:, 0:nch, :],
                                                in1=xrg3[:, 0:nch, :], op=OP.add)
                        # leaky -> odd halves
                        nc.scalar.activation(out=gT3[:, 0:nch, 64:128],
                                             in_=gT3[:, 0:nch, 0:64],
                                             func=AF.Prelu, alpha=NEG)
                        # * att -> even halves
                        nc.vector.tensor_tensor(out=gT3[:, 0:nch, 0:64],
                                                in0=gT3[:, 0:nch, 64:128],
                                                in1=at[:].rearrange(
                                                    "p (one f) -> p one f", one=1)
                                                    .to_broadcast([128, nch, 64])[:],
                                                op=OP.mult)
                        # logits
                        nc.vector.tensor_reduce(out=exl[:, 0:nch],
                                                in_=gT3[:, 0:nch, 0:64],
                                                axis=mybir.AxisListType.X, op=OP.add)
                        nc.vector.tensor_scalar(out=exl[:, 0:nch], in0=exl[:, 0:nch],
                                                scalar1=60.0, scalar2=None, op0=OP.min)
                        nc.scalar.activation(out=exl[:, 0:nch], in_=exl[:, 0:nch],
                                             func=AF.Exp)
                        # vals (ex*xlg | ex) into gT cols [0:65*nch]
                        from concourse.bass_types import AP as _AP
                        g0 = gT[:]
                        vbody = _AP(g0.tensor, g0.offset,
                                    [list(g0.ap[0]), [65, nch], [1, 64]])
                        vex = _AP(g0.tensor, g0.offset + 64,
                                  [list(g0.ap[0]), [65, nch], [1, 1]])
                        nc.vector.tensor_tensor(
                            out=vbody,
                            in0=xlg3[:, 0:nch, :],
                            in1=exl[:, 0:nch].rearrange("p (c one) -> p c one", one=1)
                                .to_broadcast([128, nch, 64])[:],
                            op=OP.mult)
                        nc.vector.tensor_copy(out=vex,
                                              in_=exl[:, 0:nch].rearrange(
                                                  "p (c one) -> p c one", one=1)[:])
                    nd = pD.tile([128, 65], F32, tag="nd")
                    for c in range(nch if not _skip_edge else 1):
                        nc.tensor.matmul(out=nd[:], lhsT=S_all[:, 128 * c:128 * (c + 1)],
                                         rhs=gT[:, 65 * c:65 * (c + 1)],
                                         start=(c == 0), stop=(c == nch - 1))
                    nds = sc.tile([128, 65], F32, tag="nds")
                    nc.scalar.copy(out=nds[:], in_=nd[:])
                    den = sc.tile([128, 1], F32, tag="den")
                    nc.vector.tensor_scalar(out=den[:], in0=nds[:, 64:65], scalar1=1e-30,
                                            scalar2=None, op0=OP.add)
                    rden = sc.tile([128, 1], F32, tag="rden")
                    nc.vector.reciprocal(out=rden[:], in_=den[:])
                    hb = sc.tile([128, 64], F32, tag="hb")
                    nc.vector.tensor_scalar(out=hb[:], in0=nds[:, 0:64], scalar1=rden[:],
                                            scalar2=None, op0=OP.mult)
                    nc.vector.tensor_tensor(out=hb[:], in0=hb[:],
                                            in1=res_own[:, 64 * b:64 * (b + 1)], op=OP.add)
                    nc.vector.tensor_tensor(out=hb[:], in0=hb[:], in1=bs[:], op=OP.add)
                    hf = sc.tile([128, 64], F32, tag="hf")
                    nc.scalar.activation(out=hf[:], in_=hb[:],
                                         func=AF.Relu if not last else AF.Copy)
                    if not last:
                        ph = pA.tile([128, 512], F32, tag="t512")
                        nc.tensor.transpose(out=ph[0:64, 0:128], in_=hf[:], identity=id_f[:])
                        nc.scalar.copy(out=hT[0:64, 128 * b:128 * (b + 1)], in_=ph[0:64, 0:128])
                    else:
                        B = sc.tile([128, 256], F32, tag="B")
                        nc.vector.tensor_tensor(
                            out=B[:], in0=bl_sb[:, b:b + 1].to_broadcast([128, 256])[:],
                            in1=iota2_f[:], op=OP.is_equal)
                        for i in range(2):
                            pp = pC.tile([128, 512], F32, tag="x512")
                            nc.tensor.matmul(out=pp[:, 0:64], lhsT=B[:, 128 * i:128 * (i + 1)],
                                             rhs=hf[:], start=True, stop=True)
                            nc.vector.tensor_tensor(out=pooled[i][:], in0=pooled[i][:],
                                                    in1=pp[:, 0:64], op=OP.add)

            nc.sync.dma_start(out=outp[:, 0:64], in_=pooled[0][:])
            nc.sync.dma_start(out=outp[:, 64:128], in_=pooled[1][:])

    _legalize_waits(nc)
    _CACHE[key] = nc
    return nc


def _prep(ei):
    """Host index prep. Returns (meta, per-core input dict pieces)."""
    key = ("prep", ei.tobytes()[:4096], int(ei.sum()))
    if key in _CACHE:
        return _CACHE[key]
    E = ei.shape[1]
    src = np.concatenate([ei[0], np.arange(N, dtype=np.int64)])
    dst = np.concatenate([ei[1], np.arange(N, dtype=np.int64)])
    kk = src // RANGE                  # source core bin
    kk[E:] = NC_                       # self-loop bin (gathers own xl table)
    gb = dst >> 7                      # global 128-node block (0..391)
    order = np.lexsort((kk, gb))
    src, dst, gb, kk = src[order], dst[order], gb[order], kk[order]
    core = gb // NB
    b = gb % NB
    gid = (core * NB + b) * NBIN + kk  # per (core, block, bin) group
    cnts = np.bincount(gid, minlength=NC_ * NB * NBIN).reshape(NC_, NB, NBIN)
    caps = cnts.max(axis=0)            # [NB, 9]
    caps = ((caps + 31) // 32) * 32   # 32: keep idx slices 4B-aligned
    nvb = caps.sum(axis=1)
    nvb_pad = ((nvb + 127) // 128) * 128
    nvb_pad = np.maximum(nvb_pad, 128)
    caps[:, NBIN - 1] += nvb_pad - nvb  # absorb tail pad into last bin
    NCH = (nvb_pad // 128).astype(np.int64)
    meta = (tuple(int(x) for x in NCH), tuple(int(x) for x in caps.reshape(-1)))

    soff = np.concatenate([[0], np.cumsum(nvb_pad)])
    capoff = np.zeros((NB, NBIN), np.int64)
    acc = 0
    cbase = np.zeros((NB, NBIN), np.int64)
    for bb in range(NB):
        o = 0
        for k in range(NBIN):
            capoff[bb, k] = o
            cbase[bb, k] = acc
            o += caps[bb, k]
            acc += caps[bb, k] // 16
    TOTCOL = int(acc)
    CHTOT = int(soff[-1] // 128)

    # per-edge placement
    gstart = np.concatenate([[0], np.cumsum(cnts.reshape(-1))])
    rank = np.arange(len(src)) - np.repeat(gstart[:-1], cnts.reshape(-1))
    srcl = src % RANGE
    jv = (srcl % HALF).astype(np.uint16)
    hv = (srcl // HALF).astype(np.int8)
    dlv = (dst & 127).astype(np.int16)

    idxs, dstls, ms = [], [], []
    for c in range(NC_):
        sel = core == c
        r = rank[sel]
        bb = b[sel]
        k = kk[sel]
        idx_t = np.zeros((16, TOTCOL), np.uint16)
        idx_t[r & 15, cbase[bb, k] + (r >> 4)] = jv[sel]
        dstl_t = np.full((128, CHTOT), -1, np.int16)
        m_t = np.zeros((128, CHTOT), np.int8)
        slot = soff[bb] + capoff[bb, k] + r
        dstl_t[slot & 127, slot >> 7] = dlv[sel]
        m_t[slot & 127, slot >> 7] = hv[sel]
        idxs.append(idx_t)
        dstls.append(dstl_t)
        ms.append(m_t)
    out = (meta, idxs, dstls, ms)
    _CACHE[key] = out
    return out


def kernel(**inputs):
    inp = {k: np.asarray(v) for k, v in inputs.items()}
    x = inp["x"].astype(np.float32)
    ei = inp["edge_index"].astype(np.int64)
    batch = inp["batch"].astype(np.int64)

    meta, idxs, dstls, ms = _prep(ei)
    NCHMAX = max(meta[0])
    nc = _build_program(meta)

    xpad = np.zeros((NPAD, DIN), np.float32)
    xpad[:N] = x
    blpad = np.full(NPAD, -1, np.int64)
    blpad[:N] = batch
    in_maps = []
    for c in range(NC_):
        d = {
            "xT": np.ascontiguousarray(xpad[c * RANGE:(c + 1) * RANGE].T).astype(ml_dtypes.bfloat16),
            "idxp": idxs[c], "dstlp": dstls[c], "mp": ms[c],
            "blp": np.ascontiguousarray(
                blpad[c * RANGE:(c + 1) * RANGE].reshape(NB, 128).T.astype(np.float32)),
        }
        for li in range(3):
            dinp = DINP[li]
            wcat = np.zeros((dinp, 192), np.float32)
            wcat[:, 0:64] = inp[f"Wl{li}"].astype(np.float32).T
            wcat[:, 64:128] = inp[f"Wr{li}"].astype(np.float32).T
            wcat[:, 128:192] = inp[f"Rw{li}"].astype(np.float32).T
            d[f"w{li}"] = wcat.astype(ml_dtypes.bfloat16)
            d[f"at{li}"] = inp[f"att{li}"].astype(np.float32).reshape(1, 64)
            d[f"bs{li}"] = np.tile(
                (inp[f"b{li}"] + inp[f"Rb{li}"]).astype(np.float32), (128, 1))
        in_maps.append(d)

    res = run_bass_kernel_spmd(nc, in_maps, list(range(NC_)))
    pooled = np.zeros((NG, HID), np.float32)
    for c in range(NC_):
        o = res.results[c]["outp"]
        pooled[0:128] += o[:, 0:64]
        pooled[128:256] += o[:, 64:128]
    cnt = np.maximum(np.bincount(batch, minlength=NG), 1).astype(np.float32)
    pooled /= cnt[:, None]
    out = pooled @ inp["Wf"].astype(np.float32).T + inp["bf"].astype(np.float32)[None, :]
    kernel.last_hw_ns = 0
    return out.reshape(NG, 1).astype(np.float32)


kernel.last_hw_ns = 0


# revision 24
# speedup vs baseline: 1.4129x; 1.1181x over previous
"""GATv2 (3-layer, heads=1) fully on Trainium2, 8 NeuronCores, ONE launch.

Sharding: destination-node ranges (6272 nodes/core) -> segment softmax and
scatter-add are core-local. Per layer the small xl table ([N,64]) is
AllGathered (bf16) and each core gathers xl[src] for its edges from
feature-transposed SBUF tables via the native Pool indirect_copy
instruction (8 per-source-core tables, half-split to satisfy the ~4096
index-offset limit). xr[dst] expansion and the segment num/den reductions
are one-hot matmuls on the PE. Host does only index prep + final pooling
combine.
"""
import sys
import numpy as np
import ml_dtypes

sys.path.insert(0, "/opt/trn_rl_repo")

import concourse.bass as bass
import concourse.mybir as mybir
from concourse.tile import TileContext
from concourse.masks import make_identity
import concourse.tile_sem_assignment as _tsa
# this walrus build allows very few semaphore waits per instruction;
# use few DMA sem lanes and legalize the rest onto NOP chains below.
_tsa.NUM_SWDGE_GLOBAL_SEMS = 2
_tsa.NUM_HWDGE_SEMS = 2
from concourse.bass_utils import run_bass_kernel_spmd

F32 = mybir.dt.float32
BF16 = mybir.dt.bfloat16
I16 = mybir.dt.int16
I8 = mybir.dt.int8
U16 = mybir.dt.uint16
AF = mybir.ActivationFunctionType
OP = mybir.AluOpType

NC_ = 8
N = 50000
DIN = 128
HID = 64
NG = 256
NEG = 0.2
RANGE = 6272              # dst nodes per core
NPAD = RANGE * NC_        # 50176
NB = RANGE // 128         # 49 blocks per core
HALF = RANGE // 2         # 3136 table columns (half-split)
NBIN = NC_ + 1            # 8 source-core bins + 1 self-loop bin
DINP = [DIN, HID, HID]


def _legalize_waits(nc, keep=1, nop_cap=1):
    """Move excess semaphore waits onto chained same-engine NOPs."""
    cnt = [0]

    def mknop(engine, waits):
        cnt[0] += 1
        n = mybir.InstNoOp(name=f"lgl-{cnt[0]}", ins=[], outs=[])
        n.engine = engine
        n.sync_info = mybir.SyncInfo(on_wait=list(waits), on_update=[])
        try:
            nc.register_instruction(n)
        except Exception:
            pass
        return n

    for bbname, bassbb in nc.bb_map.items():
        bb = bassbb.bb
        insts = bb.instructions
        out = []
        for inst in insts:
            si = inst.sync_info
            waits = list(si.on_wait) if si is not None else []
            if len(waits) > keep:
                excess, kept = waits[:-keep], waits[-keep:]
                for i in range(0, len(excess), nop_cap):
                    out.append(mknop(inst.engine, excess[i:i + nop_cap]))
                inst.sync_info = mybir.SyncInfo(on_wait=kept,
                                                on_update=list(si.on_update))
            out.append(inst)
        if len(out) != len(insts):
            bb.instructions = out


_CACHE = {}


def _build_program(meta):
    """meta = (tuple NCH[49], tuple caps[49*8]) — static across cores."""
    import os
    _skip_gather = os.environ.get("GAT_SKIP_GATHER") == "1"
    _skip_edge = os.environ.get("GAT_SKIP_EDGE") == "1"
    _nlayers = int(os.environ.get("GAT_LAYERS", "3"))
    key = ("prog", meta, _skip_gather, _skip_edge, _nlayers)
    if key in _CACHE:
        return _CACHE[key]
    NCH = list(meta[0])
    caps = np.asarray(meta[1], np.int64).reshape(NB, NBIN)
    NVb = [n * 128 for n in NCH]
    CHTOT = sum(NCH)
    TOTCOL = sum(NVb) // 16
    NCHMAX = max(NCH)
    # slot/idx col offsets
    soff = np.concatenate([[0], np.cumsum(NVb)])       # slot offset per block
    cb = np.zeros((NB, NBIN), np.int64)                # idx col base per call
    acc = 0
    for b in range(NB):
        for k in range(NBIN):
            cb[b, k] = acc
            acc += caps[b, k] // 16
    assert acc == TOTCOL

    nc = bass.Bass(num_devices=NC_)
    xT = nc.declare_dram_parameter("xT", [DIN, RANGE], BF16, isOutput=False)
    idxp = nc.declare_dram_parameter("idxp", [16, TOTCOL], U16, isOutput=False)
    dstlp = nc.declare_dram_parameter("dstlp", [128, CHTOT], I16, isOutput=False)
    mp = nc.declare_dram_parameter("mp", [128, CHTOT], I8, isOutput=False)
    blp = nc.declare_dram_parameter("blp", [128, NB], F32, isOutput=False)
    wps, atps, bps = [], [], []
    for li in range(3):
        wps.append(nc.declare_dram_parameter(f"w{li}", [DINP[li], 192], BF16, isOutput=False))
        atps.append(nc.declare_dram_parameter(f"at{li}", [1, 64], F32, isOutput=False))
        bps.append(nc.declare_dram_parameter(f"bs{li}", [128, 64], F32, isOutput=False))
    outp = nc.declare_dram_parameter("outp", [128, 128], F32, isOutput=True)
    spe_d = nc.dram_tensor("spe_d", [128, NB * NCHMAX * 128], F32)

    with TileContext(nc) as tc:
        with tc.tile_pool(name="wp", bufs=1) as wp, \
             tc.tile_pool(name="wl", bufs=1) as wl, \
             tc.tile_pool(name="gp", bufs=2) as gp, \
             tc.tile_pool(name="bk", bufs=1) as bk, \
             tc.tile_pool(name="sc", bufs=2) as sc, \
             tc.tile_pool(name="pA", bufs=2, space="PSUM") as pA, \
             tc.tile_pool(name="pB", bufs=2, space="PSUM") as pB, \
             tc.tile_pool(name="pC", bufs=2, space="PSUM") as pC, \
             tc.tile_pool(name="pD", bufs=2, space="PSUM") as pD, \
             tc.tile_pool(name="dr", bufs=2, space="DRAM") as dr:

            # ---- persistent tiles ----
            iota_i = wp.tile([128, 128], mybir.dt.int32, tag="ioi")
            nc.gpsimd.iota(iota_i[:], pattern=[[1, 128]], base=0, channel_multiplier=0)
            iota_f = wp.tile([128, 128], F32, tag="iof")
            nc.vector.tensor_copy(out=iota_f[:], in_=iota_i[:])
            iota2_i = wp.tile([128, 256], mybir.dt.int32, tag="io2i")
            nc.gpsimd.iota(iota2_i[:], pattern=[[1, 256]], base=0, channel_multiplier=0)
            iota2_f = wp.tile([128, 256], F32, tag="io2f")
            nc.vector.tensor_copy(out=iota2_f[:], in_=iota2_i[:])
            id_f = wp.tile([128, 128], F32, tag="idf")
            make_identity(nc, id_f[:])
            ones1 = wp.tile([1, 128], F32, tag="ones1")
            nc.vector.memset(ones1[:], 1.0)
            id_b = wp.tile([128, 128], BF16, tag="idb")
            make_identity(nc, id_b[:])

            idx_sb = wp.tile([128, TOTCOL], U16, tag="idx")
            for g in range(8):
                nc.sync.dma_start(out=idx_sb[16 * g:16 * (g + 1), :], in_=idxp[:, :])
            dstl_sb = wp.tile([128, CHTOT], I16, tag="dstl")
            nc.sync.dma_start(out=dstl_sb[:], in_=dstlp[:, :])
            m_sb = wp.tile([128, CHTOT], I8, tag="m")
            nc.sync.dma_start(out=m_sb[:], in_=mp[:, :])
            bl_sb = wp.tile([128, NB], F32, tag="bl")
            nc.sync.dma_start(out=bl_sb[:], in_=blp[:, :])

            hT = wp.tile([128, RANGE], BF16, tag="hT")
            nc.sync.dma_start(out=hT[:], in_=xT[:, :])
            xr_own = wp.tile([128, NB * 64], F32, tag="xro")
            res_own = wp.tile([128, NB * 64], F32, tag="rso")
            xlT_own = wp.tile([64, RANGE], BF16, tag="xlt")
            tbls = [wp.tile([128, HALF], BF16, tag=f"tbl{k}", name=f"tbl{k}")
                    for k in range(NC_)]
            tbl_self = wp.tile([128, HALF], BF16, tag="tblS", name="tblS")
            tbls.append(tbl_self)
            pooled = [wp.tile([128, 64], F32, tag=f"pl{i}", name=f"pl{i}")
                      for i in range(2)]
            nc.vector.memset(pooled[0][:], 0.0)
            nc.vector.memset(pooled[1][:], 0.0)

            for li in range(_nlayers):
                dinp = DINP[li]
                last = li == _nlayers - 1
                wc = wl.tile([dinp, 192], BF16, tag="wc")
                nc.sync.dma_start(out=wc[:], in_=wps[li][:, :])
                at1 = wl.tile([1, 64], F32, tag="at1")
                nc.sync.dma_start(out=at1[:], in_=atps[li][:, :])
                pat = pC.tile([128, 512], F32, tag="x512")
                nc.tensor.matmul(out=pat[:, 0:64], lhsT=ones1[:], rhs=at1[:],
                                 start=True, stop=True)
                at = wl.tile([128, 64], F32, tag="at")
                nc.scalar.copy(out=at[:], in_=pat[:, 0:64])
                bs = wl.tile([128, 64], F32, tag="bs")
                nc.sync.dma_start(out=bs[:], in_=bps[li][:, :])

                # ---- node linear: [xl|xr|res] = h @ [Wl^T|Wr^T|Rw^T] ----
                for b in range(NB):
                    lhs = hT[0:dinp, 128 * b:128 * (b + 1)]
                    p1 = pA.tile([128, 512], F32, tag="t512")
                    nc.tensor.matmul(out=p1[:, 0:128], lhsT=lhs, rhs=wc[:, 0:128],
                                     start=True, stop=True)
                    p2 = pC.tile([128, 512], F32, tag="x512")
                    nc.tensor.matmul(out=p2[:, 0:64], lhsT=lhs, rhs=wc[:, 128:192],
                                     start=True, stop=True)
                    nc.scalar.copy(out=xr_own[:, 64 * b:64 * (b + 1)], in_=p1[:, 64:128])
                    nc.scalar.copy(out=res_own[:, 64 * b:64 * (b + 1)], in_=p2[:, 0:64])
                    xlb = sc.tile([128, 64], BF16, tag="xlb")
                    nc.scalar.copy(out=xlb[:], in_=p1[:, 0:64])
                    pxt = pB.tile([128, 1024], BF16, tag="b1024")
                    nc.tensor.transpose(out=pxt[0:64, 0:128], in_=xlb[:], identity=id_b[:])
                    nc.scalar.copy(out=xlT_own[:, 128 * b:128 * (b + 1)], in_=pxt[0:64, 0:128])

                # ---- AllGather xl (bf16) and fill gather tables ----
                cc_in = dr.tile([64, RANGE], BF16, tag="cci")
                nc.sync.dma_start(out=cc_in[:], in_=xlT_own[:])
                cc_out = dr.tile([64 * NC_, RANGE], BF16, tag="cco", addr_space="Shared")
                nc.gpsimd.collective_compute(
                    "AllGather", OP.bypass,
                    replica_groups=[list(range(NC_))],
                    ins=[cc_in[:].opt()], outs=[cc_out[:].opt()])
                for k in range(NC_):
                    nc.sync.dma_start(out=tbls[k][0:64, :],
                                      in_=cc_out[64 * k:64 * k + 64, 0:HALF])
                    nc.sync.dma_start(out=tbls[k][64:128, :],
                                      in_=cc_out[64 * k:64 * k + 64, HALF:RANGE])
                nc.sync.dma_start(out=tbl_self[0:64, :], in_=xlT_own[:, 0:HALF])
                nc.sync.dma_start(out=tbl_self[64:128, :], in_=xlT_own[:, HALF:RANGE])

                # ---- edge phase, per dst block ----
                for b in range(NB):
                    nch = NCH[b]
                    nv = NVb[b]
                    gat = gp.tile([128, nv], BF16, tag="g")
                    off = 0
                    for k in range(NBIN):
                        cap = int(caps[b, k])
                        if cap == 0 or _skip_gather:
                            continue
                        nc.gpsimd.indirect_copy(
                            out=gat[:, off:off + cap], data=tbls[k][:],
                            idxs=idx_sb[:, int(cb[b, k]):int(cb[b, k]) + cap // 16],
                            i_know_ap_gather_is_preferred=True)
                        off += cap
                    ch0 = int(soff[b]) // 128
                    dstl_f = bk.tile([128, NCHMAX], F32, tag="dstlf")
                    nc.vector.tensor_copy(out=dstl_f[:, 0:nch],
                                          in_=dstl_sb[:, ch0:ch0 + nch])

                    S_all = bk.tile([128, NCHMAX * 128], F32, tag="sall")
                    gT = bk.tile([128, NCHMAX * 128], F32, tag="gt")
                    xlg = bk.tile([128, NCHMAX * 64], F32, tag="xlg")
                    xrg = bk.tile([128, NCHMAX * 64], F32, tag="xrg")
                    exl = bk.tile([128, NCHMAX], F32, tag="exl")
                    if not _skip_edge:
                        # one-hot S_ep for all chunks in ONE op:
                        # S[p, c, j] = (dstl[p, c] == iota[j])
                        nc.vector.tensor_tensor(
                            out=S_all[:].rearrange("p (c j) -> p c j", c=NCHMAX)[:, 0:nch, :],
                            in0=dstl_f[:, 0:nch].rearrange("p (c one) -> p c one", one=1)
                                .to_broadcast([128, nch, 128])[:],
                            in1=iota_f[:].rearrange("p (one j) -> p one j", one=1)
                                .to_broadcast([128, nch, 128])[:],
                            op=OP.is_equal)
                    for G0 in range(0, nch if not _skip_edge else 0, 8):
                        GN = min(8, nch - G0)
                        pgt = pB.tile([128, 1024], BF16, tag="b1024")
                        for j in range(GN):
                            c = G0 + j
                            nc.tensor.transpose(out=pgt[:, 128 * j:128 * (j + 1)],
                                                in_=gat[:, 128 * c:128 * (c + 1)],
                                                identity=id_b[:])
                        nc.scalar.copy(out=gT[:, 128 * G0:128 * (G0 + GN)],
                                       in_=pgt[:, 0:128 * GN])
                    spb = b * NCHMAX * 128
                    if li > 0 and not _skip_edge:
                        speB = bk.tile([128, NCHMAX * 128], F32, tag="speB")
                        nc.sync.dma_start(out=speB[:, 0:nv],
                                          in_=spe_d[:, spb:spb + nv])
                    for G0 in range(0, nch if not _skip_edge else 0, 8):
                        GN = min(8, nch - G0)
                        pxr = pC.tile([128, 512], F32, tag="x512")
                        if li == 0:
                            for g0 in range(G0, G0 + GN, 4):
                                gn = min(4, G0 + GN - g0)
                                ps = pA.tile([128, 512], F32, tag="t512")
                                for j in range(gn):
                                    c = g0 + j
                                    nc.tensor.transpose(
                                        out=ps[:, 128 * j:128 * (j + 1)],
                                        in_=S_all[:, 128 * c:128 * (c + 1)],
                                        identity=id_f[:])
                                spe = sc.tile([128, 512], F32, tag="spe")
                                nc.scalar.copy(out=spe[:, 0:128 * gn],
                                               in_=ps[:, 0:128 * gn])
                                nc.sync.dma_start(
                                    out=spe_d[:, spb + 128 * g0:spb + 128 * (g0 + gn)],
                                    in_=spe[:, 0:128 * gn])
                                for j in range(gn):
                                    jj = g0 - G0 + j
                                    nc.tensor.matmul(out=pxr[:, 64 * jj:64 * (jj + 1)],
                                                     lhsT=spe[:, 128 * j:128 * (j + 1)],
                                                     rhs=xr_own[:, 64 * b:64 * (b + 1)],
                                                     start=True, stop=True)
                        else:
                            for j in range(GN):
                                c = G0 + j
                                nc.tensor.matmul(out=pxr[:, 64 * j:64 * (j + 1)],
                                                 lhsT=speB[:, 128 * c:128 * (c + 1)],
                                                 rhs=xr_own[:, 64 * b:64 * (b + 1)],
                                                 start=True, stop=True)
                        nc.scalar.copy(out=xrg[:, 64 * G0:64 * (G0 + GN)],
                                       in_=pxr[:, 0:64 * GN])
                    if not _skip_edge:
                        gT3 = gT[:].rearrange("p (c f) -> p c f", c=NCHMAX)
                        xlg3 = xlg[:].rearrange("p (c f) -> p c f", c=NCHMAX)
                        xrg3 = xrg[:].rearrange("p (c f) -> p c f", c=NCHMAX)
                        m3 = m_sb[:, ch0:ch0 + nch].rearrange(
                            "p (c one) -> p c one", one=1).to_broadcast([128, nch, 64])
                        # half-select xlg from gT (even/odd 64-col halves by m)
                        nc.vector.tensor_copy(out=xlg3[:, 0:nch, :],
                                              in_=gT3[:, 0:nch, 0:64])
                        nc.vector.copy_predicated(out=xlg3[:, 0:nch, :], mask=m3[:],
                                                  data=gT3[:, 0:nch, 64:128])
                        # e = xlg + xrg -> gT even halves
                        nc.vector.tensor_tensor(out=gT3[:, 0:nch, 0:64],
                                                in0=xlg3[:, 0:nch, :],
                                                in1=xrg3[:, 0:nch, :], op=OP.add)
                        # leaky -> odd halves
                        nc.scalar.activation(out=gT3[:, 0:nch, 64:128],
                                             in_=gT3[:, 0:nch, 0:64],
                                             func=AF.Prelu, alpha=NEG)
                        # * att -> even halves
                        nc.vector.tensor_tensor(out=gT3[:, 0:nch, 0:64],
                                                in0=gT3[:, 0:nch, 64:128],
                                                in1=at[:].rearrange(
                                                    "p (one f) -> p one f", one=1)
                                                    .to_broadcast([128, nch, 64])[:],
                                                op=OP.mult)
                        # logits
                        nc.vector.tensor_reduce(out=exl[:, 0:nch],
                                                in_=gT3[:, 0:nch, 0:64],
                                                axis=mybir.AxisListType.X, op=OP.add)
                        nc.vector.tensor_scalar(out=exl[:, 0:nch], in0=exl[:, 0:nch],
                                                scalar1=60.0, scalar2=None, op0=OP.min)
                        nc.scalar.activation(out=exl[:, 0:nch], in_=exl[:, 0:nch],
                                             func=AF.Exp)
                        # vals (ex*xlg | ex) into gT cols [0:65*nch]
                        from concourse.bass_types import AP as _AP
                        g0 = gT[:]
                        vbody = _AP(g0.tensor, g0.offset,
                                    [list(g0.ap[0]), [65, nch], [1, 64]])
                        vex = _AP(g0.tensor, g0.offset + 64,
                                  [list(g0.ap[0]), [65, nch], [1, 1]])
                        nc.vector.tensor_tensor(
                            out=vbody,
                            in0=xlg3[:, 0:nch, :],
                            in1=exl[:, 0:nch].rearrange("p (c one) -> p c one", one=1)
                                .to_broadcast([128, nch, 64])[:],
                            op=OP.mult)
                        nc.vector.tensor_copy(out=vex,
                                              in_=exl[:, 0:nch].rearrange(
                                                  "p (c one) -> p c one", one=1)[:])
                    nd = pD.tile([128, 65], F32, tag="nd")
                    for c in range(nch if not _skip_edge else 1):
                        nc.tensor.matmul(out=nd[:], lhsT=S_all[:, 128 * c:128 * (c + 1)],
                                         rhs=gT[:, 65 * c:65 * (c + 1)],
                                         start=(c == 0), stop=(c == nch - 1))
                    nds = sc.tile([128, 65], F32, tag="nds")
                    nc.scalar.copy(out=nds[:], in_=nd[:])
                    den = sc.tile([128, 1], F32, tag="den")
                    nc.vector.tensor_scalar(out=den[:], in0=nds[:, 64:65], scalar1=1e-30,
                                            scalar2=None, op0=OP.add)
                    rden = sc.tile([128, 1], F32, tag="rden")
                    nc.vector.reciprocal(out=rden[:], in_=den[:])
                    hb = sc.tile([128, 64], F32, tag="hb")
                    nc.vector.tensor_scalar(out=hb[:], in0=nds[:, 0:64], scalar1=rden[:],
                                            scalar2=None, op0=OP.mult)
                    nc.vector.tensor_tensor(out=hb[:], in0=hb[:],
                                            in1=res_own[:, 64 * b:64 * (b + 1)], op=OP.add)
                    nc.vector.tensor_tensor(out=hb[:], in0=hb[:], in1=bs[:], op=OP.add)
                    hf = sc.tile([128, 64], F32, tag="hf")
                    nc.scalar.activation(out=hf[:], in_=hb[:],
                                         func=AF.Relu if not last else AF.Copy)
                    if not last:
                        ph = pA.tile([128, 512], F32, tag="t512")
                        nc.tensor.transpose(out=ph[0:64, 0:128], in_=hf[:], identity=id_f[:])
                        nc.scalar.copy(out=hT[0:64, 128 * b:128 * (b + 1)], in_=ph[0:64, 0:128])
                    else:
                        B = sc.tile([128, 256], F32, tag="B")
                        nc.vector.tensor_tensor(
                            out=B[:], in0=bl_sb[:, b:b + 1].to_broadcast([128, 256])[:],
                            in1=iota2_f[:], op=OP.is_equal)
                        for i in range(2):
                            pp = pC.tile([128, 512], F32, tag="x512")
                            nc.tensor.matmul(out=pp[:, 0:64], lhsT=B[:, 128 * i:128 * (i + 1)],
                                             rhs=hf[:], start=True, stop=True)
                            nc.vector.tensor_tensor(out=pooled[i][:], in0=pooled[i][:],
                                                    in1=pp[:, 0:64], op=OP.add)

            nc.sync.dma_start(out=outp[:, 0:64], in_=pooled[0][:])
            nc.sync.dma_start(out=outp[:, 64:128], in_=pooled[1][:])

    _legalize_waits(nc)
    _CACHE[key] = nc
    return nc


def _prep(ei):
    """Host index prep. Returns (meta, per-core input dict pieces)."""
    key = ("prep", ei.tobytes()[:4096], int(ei.sum()))
    if key in _CACHE:
        return _CACHE[key]
    E = ei.shape[1]
    src = np.concatenate([ei[0], np.arange(N, dtype=np.int64)])
    dst = np.concatenate([ei[1], np.arange(N, dtype=np.int64)])
    kk = src // RANGE                  # source core bin
    kk[E:] = NC_                       # self-loop bin (gathers own xl table)
    gb = dst >> 7                      # global 128-node block (0..391)
    order = np.lexsort((kk, gb))
    src, dst, gb, kk = src[order], dst[order], gb[order], kk[order]
    core = gb // NB
    b = gb % NB
    gid = (core * NB + b) * NBIN + kk  # per (core, block, bin) group
    cnts = np.bincount(gid, minlength=NC_ * NB * NBIN).reshape(NC_, NB, NBIN)
    caps = cnts.max(axis=0)            # [NB, 9]
    caps = ((caps + 31) // 32) * 32   # 32: keep idx slices 4B-aligned
    nvb = caps.sum(axis=1)
    nvb_pad = ((nvb + 127) // 128) * 128
    nvb_pad = np.maximum(nvb_pad, 128)
    caps[:, NBIN - 1] += nvb_pad - nvb  # absorb tail pad into last bin
    NCH = (nvb_pad // 128).astype(np.int64)
    meta = (tuple(int(x) for x in NCH), tuple(int(x) for x in caps.reshape(-1)))

    soff = np.concatenate([[0], np.cumsum(nvb_pad)])
    capoff = np.zeros((NB, NBIN), np.int64)
    acc = 0
    cbase = np.zeros((NB, NBIN), np.int64)
    for bb in range(NB):
        o = 0
        for k in range(NBIN):
            capoff[bb, k] = o
            cbase[bb, k] = acc
            o += caps[bb, k]
            acc += caps[bb, k] // 16
    TOTCOL = int(acc)
    CHTOT = int(soff[-1] // 128)

    # per-edge placement
    gstart = np.concatenate([[0], np.cumsum(cnts.reshape(-1))])
    rank = np.arange(len(src)) - np.repeat(gstart[:-1], cnts.reshape(-1))
    srcl = src % RANGE
    jv = (srcl % HALF).astype(np.uint16)
    hv = (srcl // HALF).astype(np.int8)
    dlv = (dst & 127).astype(np.int16)

    idxs, dstls, ms = [], [], []
    for c in range(NC_):
        sel = core == c
        r = rank[sel]
        bb = b[sel]
        k = kk[sel]
        idx_t = np.zeros((16, TOTCOL), np.uint16)
        idx_t[r & 15, cbase[bb, k] + (r >> 4)] = jv[sel]
        dstl_t = np.full((128, CHTOT), -1, np.int16)
        m_t = np.zeros((128, CHTOT), np.int8)
        slot = soff[bb] + capoff[bb, k] + r
        dstl_t[slot & 127, slot >> 7] = dlv[sel]
        m_t[slot & 127, slot >> 7] = hv[sel]
        idxs.append(idx_t)
        dstls.append(dstl_t)
        ms.append(m_t)
    out = (meta, idxs, dstls, ms)
    _CACHE[key] = out
    return out


def kernel(**inputs):
    inp = {k: np.asarray(v) for k, v in inputs.items()}
    x = inp["x"].astype(np.float32)
    ei = inp["edge_index"].astype(np.int64)
    batch = inp["batch"].astype(np.int64)

    meta, idxs, dstls, ms = _prep(ei)
    NCHMAX = max(meta[0])
    nc = _build_program(meta)

    xpad = np.zeros((NPAD, DIN), np.float32)
    xpad[:N] = x
    blpad = np.full(NPAD, -1, np.int64)
    blpad[:N] = batch
    in_maps = []
    for c in range(NC_):
        d = {
            "xT": np.ascontiguousarray(xpad[c * RANGE:(c + 1) * RANGE].T).astype(ml_dtypes.bfloat16),
            "idxp": idxs[c], "dstlp": dstls[c], "mp": ms[c],
            "blp": np.ascontiguousarray(
                blpad[c * RANGE:(c + 1) * RANGE].reshape(NB, 128).T.astype(np.float32)),
        }
        for li in range(3):
            dinp = DINP[li]
            wcat = np.zeros((dinp, 192), np.float32)
            wcat[:, 0:64] = inp[f"Wl{li}"].astype(np.float32).T
            wcat[:, 64:128] = inp[f"Wr{li}"].astype(np.float32).T
            wcat[:, 128:192] = inp[f"Rw{li}"].astype(np.float32).T
            d[f"w{li}"] = wcat.astype(ml_dtypes.bfloat16)
            d[f"at{li}"] = inp[f"att{li}"].astype(np.float32).reshape(1, 64)
            d[f"bs{li}"] = np.tile(
                (inp[f"b{li}"] + inp[f"Rb{li}"]).astype(np.float32), (128, 1))
        in_maps.append(d)

    res = run_bass_kernel_spmd(nc, in_maps, list(range(NC_)))
    pooled = np.zeros((NG, HID), np.float32)
    for c in range(NC_):
        o = res.results[c]["outp"]
        pooled[0:128] += o[:, 0:64]
        pooled[128:256] += o[:, 64:128]
    cnt = np.maximum(np.bincount(batch, minlength=NG), 1).astype(np.float32)
    pooled /= cnt[:, None]
    out = pooled @ inp["Wf"].astype(np.float32).T + inp["bf"].astype(np.float32)[None, :]
    kernel.last_hw_ns = 0
    return out.reshape(NG, 1).astype(np.float32)


kernel.last_hw_ns = 0
